# revision 1
# baseline (speedup 1.0000x reference)
"""Trainium2 Bass kernel for nn_CorrTrajBlock (sparse_attention).

Data-parallel over batch B=8 across 8 NeuronCores; one sample per core.

Per-core pipeline (C=512, T=8, H=W=28, HW=784, S=T*HW=6272, R=64, K=4,
Cq=128, P=T*R=512):
  1. template_p = w_reduce_eff @ x[:, 0]           (fp32 matmul, 64x784)
     spt_inds   = argmax over HW                   (DVE max/max_index)
  2. template_resample = gather cols of x frame 0  (dma_gather from x_sc)
  3. affinity = template_resample^T @ x_flat       (fp32 matmul, 64x6272)
     topk4 per (r, t) over HW                      (DVE max/max_index)
  4. traj gather (2048 rows of x_sc), PE-transpose to (c, ktr) bf16
     points = sum_k traj (fp32), transposed to (c, p), bounced to fp32r
  5. fuse = w_proj_eff @ [traj; coords] (bf16 mm) -> max over k -> +bias2
     tc = relu(conv_t(fuse) + bias3)  (bf16 mm over 3 taps)
  6. zT[s, p] = x^T @ points  (fp32r mm, per 128-row s-tile)
     softmax over p in-tile (DVE reduce_max -> ACT exp+accum -> scale)
     PE-transpose to proj (p, s) bf16
  7. prop = tcT^T @ proj + I @ x  (bf16 + fp32r mm into same PSUM)
     out = copy PSUM -> SBUF -> DRAM
"""
import sys

sys.path.insert(0, "/opt/trn_rl_repo")

import numpy as np
import concourse.bass as bass
import concourse.mybir as mybir
import concourse.tile as tile
from concourse import bacc
from concourse.bass_utils import run_bass_kernel_spmd

F32 = mybir.dt.float32
F32R = mybir.dt.float32r
BF16 = mybir.dt.bfloat16
I16 = mybir.dt.int16
I32 = mybir.dt.int32
U32 = mybir.dt.uint32
AF = mybir.ActivationFunctionType
ALU = mybir.AluOpType
AX = mybir.AxisListType

B, C, T, H, W = 8, 512, 8, 28, 28
HW = H * W            # 784
S = T * HW            # 6272
R = 64
K = 4
Cq = 128
P = T * R             # 512
CC = C // 128         # 4
NST = S // 128        # 49 s-tiles
NCH = 13              # s-chunks: 12 x 512 + 1 x 128

_CACHED = {}


def build_nc():
    nc = bacc.Bacc("TRN2", debug=False)

    X_CS = nc.dram_tensor("x_cs", [C, S], F32, kind="ExternalInput").ap()
    X_SC = nc.dram_tensor("x_sc", [S, C], F32, kind="ExternalInput").ap()
    WRT = nc.dram_tensor("wrT", [C, R], F32, kind="ExternalInput").ap()
    WPT = nc.dram_tensor("wpT", [C, Cq], BF16, kind="ExternalInput").ap()
    WPC = nc.dram_tensor("wpc", [2, Cq], BF16, kind="ExternalInput").ap()
    WTT = nc.dram_tensor("wtT", [3, Cq, C], BF16, kind="ExternalInput").ap()
    B2 = nc.dram_tensor("b2", [Cq, 1], F32, kind="ExternalInput").ap()
    B3 = nc.dram_tensor("b3", [CC, 128], F32, kind="ExternalInput").ap()
    IDF = nc.dram_tensor("identf", [128, 128], F32, kind="ExternalInput").ap()
    IDB = nc.dram_tensor("identbf", [128, 128], BF16, kind="ExternalInput").ap()
    IDR = nc.dram_tensor("identr", [128, 128], F32R, kind="ExternalInput").ap()
    OUT = nc.dram_tensor("out", [C, S], F32, kind="ExternalOutput").ap()

    # DRAM scratch (declared as outputs while debugging races)
    DBG = True
    kind = dict(kind="ExternalOutput") if DBG else {}
    TIDX = nc.dram_tensor("tidx_scr", [128, 1], I16, **kind).ap()
    GIDX = nc.dram_tensor("gidx_scr", [2048], I16, **kind).ap()
    CROW = nc.dram_tensor("crow_scr", [2048], BF16, **kind).ap()
    CCOL = nc.dram_tensor("ccol_scr", [2048], BF16, **kind).ap()
    PTS = nc.dram_tensor("pts_scr", [128, CC, P], F32R, **kind).ap()
    DTC = (nc.dram_tensor("dbg_tc", [128, CC, P], BF16, kind="ExternalOutput").ap()
           if DBG else None)

    Xr = X_CS.rearrange("(cc p) s -> p cc s", p=128)
    OUTr = OUT.rearrange("(cc p) s -> p cc s", p=128)

    with tile.TileContext(nc) as tc:
        import contextlib
        ctx = contextlib.ExitStack()
        pers = ctx.enter_context(tc.tile_pool(name="pers", bufs=1))
        sb = ctx.enter_context(tc.tile_pool(name="sb", bufs=2))
        ps = ctx.enter_context(tc.tile_pool(name="ps", bufs=3, space="PSUM"))
        pstp = ctx.enter_context(tc.tile_pool(name="pstp", bufs=4, space="PSUM"))

        # ---- persistent loads ----
        xc = pers.tile([128, CC, S], F32, tag="xc")
        nc.sync.dma_start(out=xc[:, :, 0:HW], in_=Xr[:, :, 0:HW])  # frame 0 first
        xcf = xc  # true fp32 for template/affinity matmuls

        wrT_t = pers.tile([128, CC, R], F32, tag="wrT")
        nc.sync.dma_start(out=wrT_t, in_=WRT.rearrange("(cc p) r -> p cc r", p=128))
        wpT_t = pers.tile([128, CC, Cq], BF16, tag="wpT")
        nc.sync.dma_start(out=wpT_t, in_=WPT.rearrange("(cc p) q -> p cc q", p=128))
        wpc_t = pers.tile([2, Cq], BF16, tag="wpc")
        nc.sync.dma_start(out=wpc_t, in_=WPC)
        wtT_t = pers.tile([128, 3, C], BF16, tag="wtT")
        nc.sync.dma_start(out=wtT_t, in_=WTT.rearrange("d p c -> p d c"))
        b2_t = pers.tile([128, 1], F32, tag="b2")
        nc.sync.dma_start(out=b2_t, in_=B2)
        b3_t = pers.tile([128, CC], F32, tag="b3")
        nc.sync.dma_start(out=b3_t, in_=B3.rearrange("cc p -> p cc"))
        idf_t = pers.tile([128, 128], F32, tag="idf")
        nc.sync.dma_start(out=idf_t, in_=IDF)
        idb_t = pers.tile([128, 128], BF16, tag="idb")
        nc.sync.dma_start(out=idb_t, in_=IDB)
        idr_t = pers.tile([128, 128], F32R, tag="idr")
        nc.sync.dma_start(out=idr_t, in_=IDR)

        # rest of x: per-t-block DMAs so affinity can stream per t
        for tb in range(1, T):
            nc.sync.dma_start(out=xc[:, :, tb * HW:(tb + 1) * HW],
                              in_=Xr[:, :, tb * HW:(tb + 1) * HW])

        # ---- phase 1: template ----
        tpl_sb = pers.tile([64, HW], F32, tag="tpl")
        for h in range(2):
            tp_ps = ps.tile([64, 392], F32, tag="acc")
            for cc in range(CC):
                nc.tensor.matmul(tp_ps, lhsT=wrT_t[:, cc, :],
                                 rhs=xcf[:, cc, h * 392:(h + 1) * 392],
                                 start=(cc == 0), stop=(cc == CC - 1))
            nc.scalar.activation(tpl_sb[:, h * 392:(h + 1) * 392], tp_ps, AF.Copy)
        tmx = pers.tile([64, 8], F32, tag="tmx")
        tmi = pers.tile([64, 8], U32, tag="tmi")
        nc.vector.max(out=tmx, in_=tpl_sb)
        nc.vector.max_index(out=tmi, in_max=tmx, in_values=tpl_sb)
        spt16 = pers.tile([64, 1], I16, tag="spt16")
        nc.vector.tensor_copy(spt16, tmi[:, 0:1])
        z64 = pers.tile([64, 1], I16, tag="z64")
        nc.vector.memset(z64, 0)
        nc.gpsimd.dma_start(out=TIDX[0:64, :], in_=spt16)
        nc.gpsimd.dma_start(out=TIDX[64:128, :], in_=z64)
        gidxT = pers.tile([128, 8], I16, tag="gidxT")
        TIDXw = TIDX.rearrange("(c p) one -> p (c one)", p=16)
        for g in range(8):
            nc.gpsimd.dma_start(out=gidxT[16 * g:16 * (g + 1), :], in_=TIDXw)
        tresT = pers.tile([128, 1, C], F32, tag="tresT")
        nc.gpsimd.dma_gather(out_ap=tresT, in_ap=X_SC, idxs_ap=gidxT,
                             num_idxs=128, num_idxs_reg=128, elem_size=C)
        tres = pers.tile([128, CC, R], F32, tag="tres")
        for cc in range(CC):
            tp = pstp.tile([128, 128], F32, tag="tp")
            nc.tensor.transpose(tp, tresT[:, 0, cc * 128:(cc + 1) * 128], idf_t)
            nc.scalar.activation(tres[:, cc, :], tp[:, 0:64], AF.Copy)

        # ---- phase 2: affinity + topk ----
        gstage = pers.tile([128, 16], I16, tag="gstage")
        fstage = pers.tile([128, 16], F32, tag="fstage")
        for pi in range(4):
            aff_sb = sb.tile([128, HW], F32, tag="aff")
            for h in range(2):
                a_ps = ps.tile([128, 392], F32, tag="acc")
                for tt in range(2):
                    t = 2 * pi + tt
                    for cc in range(CC):
                        nc.tensor.matmul(
                            a_ps[64 * tt:64 * (tt + 1), :],
                            lhsT=tres[:, cc, :],
                            rhs=xcf[:, cc, t * HW + h * 392: t * HW + (h + 1) * 392],
                            start=(cc == 0), stop=(cc == CC - 1),
                            tile_position=(0, 64 * tt))
                nc.scalar.activation(aff_sb[:, h * 392:(h + 1) * 392], a_ps, AF.Copy)
            amx = sb.tile([128, 8], F32, tag="amx")
            ami = sb.tile([128, 8], U32, tag="ami")
            nc.vector.max(out=amx, in_=aff_sb)
            nc.vector.max_index(out=ami, in_max=amx, in_values=aff_sb)
            # staging layout: [p, k*4+pi] so that dram j = 128*(4k+pi)+p
            gs_v = gstage.rearrange("p (k pi) -> p pi k", pi=4)
            fs_v = fstage.rearrange("p (k pi) -> p pi k", pi=4)
            for tt in range(2):
                t = 2 * pi + tt
                rows = slice(64 * tt, 64 * (tt + 1))
                nc.vector.tensor_scalar(gs_v[rows, pi, :],
                                        ami[rows, 0:K], float(t * HW), None,
                                        op0=ALU.add)
            nc.vector.tensor_copy(fs_v[:, pi, :], ami[:, 0:K])

        # coords: row=(i//28)/28, col=(i%28)/28  (i in [0,784))
        # floor robust to both trunc and round f32->i32 cast semantics:
        #   q1 = cast(i/28); q = q1 - (q1 > i/28)
        vq = pers.tile([128, 16], F32, tag="vq")
        nc.vector.tensor_scalar(vq, fstage, 1.0 / 28.0, None, op0=ALU.mult)
        qi = pers.tile([128, 16], I32, tag="qi")
        nc.vector.tensor_copy(qi, vq)
        qf = pers.tile([128, 16], F32, tag="qf")
        nc.vector.tensor_copy(qf, qi)
        cgt = pers.tile([128, 16], F32, tag="cgt")
        nc.vector.tensor_tensor(out=cgt, in0=qf, in1=vq, op=ALU.is_gt)
        nc.vector.tensor_tensor(out=qf, in0=qf, in1=cgt, op=ALU.subtract)
        rowb = pers.tile([128, 16], BF16, tag="rowb")
        nc.vector.tensor_scalar(rowb, qf, 1.0 / 28.0, None, op0=ALU.mult)
        colb = pers.tile([128, 16], BF16, tag="colb")
        nc.vector.scalar_tensor_tensor(colb, in0=fstage, scalar=1.0 / 28.0,
                                       in1=qf, op0=ALU.mult, op1=ALU.subtract)
        nc.gpsimd.dma_start(out=GIDX.rearrange("(q p) -> p q", p=128), in_=gstage)
        nc.gpsimd.dma_start(out=CROW.rearrange("(q p) -> p q", p=128), in_=rowb)
        nc.gpsimd.dma_start(out=CCOL.rearrange("(q p) -> p q", p=128), in_=colb)
        gidx2 = pers.tile([128, 128], I16, tag="gidx2")
        GIDXw = GIDX.rearrange("(c p) -> p c", p=16)
        for g in range(8):
            nc.gpsimd.dma_start(out=gidx2[16 * g:16 * (g + 1), :], in_=GIDXw)
        coords = pers.tile([2, K * P], BF16, tag="coords")
        nc.gpsimd.dma_start(out=coords[0:1, :], in_=CROW.rearrange("(a s) -> a s", a=1))
        nc.gpsimd.dma_start(out=coords[1:2, :], in_=CCOL.rearrange("(a s) -> a s", a=1))

        # ---- phase 3: traj gathers, fuse, points, conv ----
        # 2 rotating gather slots; pointsT accumulated in-place in s01 (tag
        # sA); scratch tags sA/sB reused for pts_r / pts_f afterwards.
        fm_f32 = pers.tile([128, P], F32, tag="fmf")
        s01 = pers.tile([128, 4, P], F32, tag="sA")
        s23 = pers.tile([128, 4, P], F32, tag="sB")
        gk = []
        for k in range(K):
            gk_t = sb.tile([128, 4, P], F32, tag="gk")
            gk.append(gk_t)
            nc.gpsimd.dma_gather(out_ap=gk_t, in_ap=X_SC,
                                 idxs_ap=gidx2[:, k * 32:(k + 1) * 32],
                                 num_idxs=512, num_idxs_reg=512, elem_size=C)
            trajk = sb.tile([128, CC, P], BF16, tag="trajk")
            for cc in range(CC):
                tp = pstp.tile([128, P], F32, tag="tp")
                for jb in range(4):
                    nc.tensor.transpose(tp[:, jb * 128:(jb + 1) * 128],
                                        gk_t[:, jb, cc * 128:(cc + 1) * 128], idf_t)
                nc.scalar.activation(trajk[:, cc, :], tp, AF.Copy)
            f_ps = ps.tile([128, P], F32, tag="acc")
            for cc in range(CC):
                nc.tensor.matmul(f_ps, lhsT=wpT_t[:, cc, :], rhs=trajk[:, cc, :],
                                 start=(cc == 0), stop=False)
            nc.tensor.matmul(f_ps, lhsT=wpc_t, rhs=coords[:, k * P:(k + 1) * P],
                             start=False, stop=True)
            if k == 0:
                nc.scalar.activation(fm_f32, f_ps, AF.Copy)
            else:
                nc.vector.tensor_tensor(out=fm_f32, in0=fm_f32, in1=f_ps, op=ALU.max)
            if k == 1:
                nc.vector.tensor_tensor(out=s01, in0=gk[0], in1=gk[1], op=ALU.add)
            if k == 3:
                nc.gpsimd.tensor_tensor(out=s23, in0=gk[2], in1=gk[3], op=ALU.add)
        fm = pers.tile([128, P], BF16, tag="fm")
        nc.vector.tensor_scalar(fm, fm_f32, b2_t, None, op0=ALU.add)

        # points = sum_k traj_k (fp32), fold 1/4 into softmax exp scale
        nc.vector.tensor_tensor(out=s01, in0=s01, in1=s23, op=ALU.add)
        pts_f = pers.tile([128, CC, P], F32, tag="sB")  # reuse s23 slot
        for cc in range(CC):
            tp = pstp.tile([128, P], F32, tag="tp")
            for jb in range(4):
                nc.tensor.transpose(tp[:, jb * 128:(jb + 1) * 128],
                                    s01[:, jb, cc * 128:(cc + 1) * 128], idf_t)
            nc.scalar.activation(pts_f[:, cc, :], tp, AF.Copy)
        nc.gpsimd.dma_start(out=PTS, in_=pts_f.bitcast(F32R))
        pts_r = pers.tile([128, CC, P], F32R, tag="sA")  # reuse s01 slot
        nc.gpsimd.dma_start(out=pts_r, in_=PTS)

        # conv over t (3 taps) + bias3 + relu -> tc bf16
        tc_bf = pers.tile([128, CC, P], BF16, tag="tcbf")
        for ct in range(CC):
            c_ps = ps.tile([128, P], F32, tag="acc")
            cs = slice(ct * 128, (ct + 1) * 128)
            nc.tensor.matmul(c_ps, lhsT=wtT_t[:, 1, cs], rhs=fm,
                             start=True, stop=False)
            nc.tensor.matmul(c_ps[:, R:P], lhsT=wtT_t[:, 0, cs], rhs=fm[:, 0:P - R],
                             start=False, stop=False)
            nc.tensor.matmul(c_ps[:, 0:P - R], lhsT=wtT_t[:, 2, cs], rhs=fm[:, R:P],
                             start=False, stop=True)
            nc.scalar.activation(tc_bf[:, ct, :], c_ps, AF.Relu,
                                 bias=b3_t[:, ct:ct + 1])
        if DTC is not None:
            nc.sync.dma_start(out=DTC, in_=tc_bf)
        tcT = pers.tile([128, CC, C], BF16, tag="tcT")
        for pb in range(4):
            tp2 = pstp.tile([128, C], BF16, tag="tp")
            for cc in range(CC):
                nc.tensor.transpose(tp2[:, cc * 128:(cc + 1) * 128],
                                    tc_bf[:, cc, pb * 128:(pb + 1) * 128], idb_t)
            nc.vector.tensor_copy(tcT[:, pb, :], tp2)

        # ---- phase 4+5: zT -> softmax -> proj transpose -> prop -> out ----
        projTP = [None] * 4
        proj_ch = None
        for st in range(NST):
            chunk, slot = st // 4, st % 4
            if slot == 0:
                projTP = []
                for _pb in range(4):
                    pjp_t = pstp.tile([128, P], BF16, tag="tp")
                    projTP.append(pjp_t)
                proj_ch = sb.tile([128, 4, P], BF16, tag="projch")
                # f32r copy of this s-chunk of x (feeds zT lhsT + identity rhs)
                cwc = min(P, S - chunk * P)
                xr_n = sb.tile([128, CC, P], F32R, tag="xr")
                nc.sync.dma_start(out=xr_n[:, :, 0:cwc],
                                  in_=Xr.bitcast(F32R)[:, :, chunk * P:chunk * P + cwc])
            z_ps = ps.tile([128, P], F32, tag="acc")
            for cc in range(CC):
                nc.tensor.matmul(z_ps,
                                 lhsT=xr_n[:, cc, slot * 128:(slot + 1) * 128],
                                 rhs=pts_r[:, cc, :],
                                 start=(cc == 0), stop=(cc == CC - 1))
            nm = sb.tile([128, 1], F32, tag="nm")
            nc.vector.tensor_reduce(nm, z_ps, axis=AX.X, op=ALU.max, negate=True)
            nm4 = sb.tile([128, 1], F32, tag="nm4")
            nc.vector.tensor_scalar(nm4, nm, 0.25, None, op0=ALU.mult)
            e_sb = sb.tile([128, P], F32, tag="esb")
            dsum = sb.tile([128, 1], F32, tag="dsum")
            nc.scalar.activation(e_sb, z_ps, AF.Exp, bias=nm4, scale=0.25,
                                 accum_out=dsum)
            rd = sb.tile([128, 1], F32, tag="rd")
            nc.vector.reciprocal(rd, dsum)
            pjT = sb.tile([128, P], BF16, tag="pjT")
            nc.vector.tensor_scalar(pjT, e_sb, rd, None, op0=ALU.mult)
            for pb in range(4):
                nc.tensor.transpose(projTP[pb][:, slot * 128:(slot + 1) * 128],
                                    pjT[:, pb * 128:(pb + 1) * 128], idb_t)
            if slot == 3 or st == NST - 1:
                cw = (slot + 1) * 128
                for pb in range(4):
                    eng = nc.scalar if pb < 2 else nc.vector
                    if pb < 2:
                        nc.scalar.activation(proj_ch[:, pb, 0:cw],
                                             projTP[pb][:, 0:cw], AF.Copy)
                    else:
                        nc.vector.tensor_copy(proj_ch[:, pb, 0:cw],
                                              projTP[pb][:, 0:cw])
                # prop for this chunk
                for ct in range(CC):
                    p_ps = ps.tile([128, cw], F32, tag="acc")
                    for pb in range(4):
                        nc.tensor.matmul(p_ps, lhsT=tcT[:, pb, ct * 128:(ct + 1) * 128],
                                         rhs=proj_ch[:, pb, 0:cw],
                                         start=(pb == 0), stop=False,
                                         skip_group_check=True)
                    nc.tensor.matmul(p_ps, lhsT=idr_t,
                                     rhs=xr_n[:, ct, 0:cw],
                                     start=False, stop=True, skip_group_check=True)
                    osb = sb.tile([128, cw], F32, tag="osb")
                    if ct % 2 == 0:
                        nc.scalar.activation(osb, p_ps, AF.Copy)
                    else:
                        nc.vector.tensor_copy(osb, p_ps)
                    nc.sync.dma_start(out=OUTr[:, ct, chunk * P:chunk * P + cw],
                                      in_=osb)
        ctx.close()
    nc.compile()
    return nc


def _host_prep(inputs):
    eps = 1e-5
    f32 = np.float32
    x = np.asarray(inputs["input"], f32)                       # (B,C,T,H,W)
    s1 = np.asarray(inputs["bn1_gamma"]) / np.sqrt(np.asarray(inputs["bn1_var"]) + eps)
    wrT = (np.asarray(inputs["w_reduce"], f32) * s1[:, None]).T.astype(f32)
    s2 = np.asarray(inputs["bn2_gamma"]) / np.sqrt(np.asarray(inputs["bn2_var"]) + eps)
    wp = np.asarray(inputs["w_proj"], f32) * s2[:, None]       # (Cq, C+2)
    b2 = (np.asarray(inputs["bn2_beta"])
          - np.asarray(inputs["bn2_mean"]) * s2).astype(f32)
    s3 = np.asarray(inputs["bn3_gamma"]) / np.sqrt(np.asarray(inputs["bn3_var"]) + eps)
    wt = np.asarray(inputs["w_t"], f32)[:, :, :, 0] * s3[:, None, None]  # (C,Cq,3)
    b3 = (np.asarray(inputs["bn3_beta"])
          - np.asarray(inputs["bn3_mean"]) * s3).astype(f32)
    bf = np.dtype("bfloat16") if hasattr(np, "bfloat16") else None
    import ml_dtypes
    bf16 = ml_dtypes.bfloat16
    common = {
        "wrT": np.ascontiguousarray(wrT),
        "wpT": np.ascontiguousarray(wp[:, :C].T.astype(bf16)),
        "wpc": np.ascontiguousarray(wp[:, C:].T.astype(bf16)),
        "wtT": np.ascontiguousarray(np.transpose(wt, (2, 1, 0)).astype(bf16)),
        "b2": b2.reshape(Cq, 1),
        "b3": b3.reshape(CC, 128),
        "identf": np.eye(128, dtype=f32),
        "identbf": np.eye(128, dtype=bf16),
        "identr": np.eye(128, dtype=f32),
    }
    x_cs = x.reshape(B, C, S)
    x_sc = np.ascontiguousarray(np.transpose(x_cs, (0, 2, 1)))
    in_maps = []
    for b in range(B):
        m = dict(common)
        m["x_cs"] = np.ascontiguousarray(x_cs[b])
        m["x_sc"] = x_sc[b]
        in_maps.append(m)
    return in_maps


def kernel(**inputs) -> np.ndarray:
    if "nc" not in _CACHED:
        _CACHED["nc"] = build_nc()
    nc = _CACHED["nc"]
    in_maps = _host_prep(inputs)
    res = run_bass_kernel_spmd(nc, in_maps, list(range(B)))
    out = np.stack([res.results[b]["out"] for b in range(B)], axis=0)
    return out.reshape(B, C, T, H, W).astype(np.float32)


if __name__ == "__main__":
    # smoke: random inputs
    rng = np.random.default_rng(0)
    pass



# revision 11
# speedup vs baseline: 1.4140x; 1.4140x over previous
"""Trainium2 Bass kernel for nn_CorrTrajBlock (sparse_attention).

Data-parallel over batch B=8 across 8 NeuronCores; one sample per core.

Per-core pipeline (C=512, T=8, H=W=28, HW=784, S=T*HW=6272, R=64, K=4,
Cq=128, P=T*R=512):
  1. template_p = w_reduce_eff @ x[:, 0]        (f32r matmul, 64x784)
     spt_inds   = argmax over HW                (DVE max/max_index)
  2. template_resample gather (64 rows of x_sc_aug), PE-transpose
  3. affinity = template_resample^T @ x_flat    (f32r matmul, 64x6272)
     topk4 per (r, t) over HW                   (DVE max/max_index)
  4. traj gather (2048 rows of x_sc_aug, coords baked in cols 512:514),
     PE-transpose to (c, ktr); points = sum_k traj (DVE adds + PE transp)
  5. fuse = w_proj_eff @ [traj; coords] (bf16) -> max over k -> +bias2
     tc = relu(conv_t(fuse) + bias3) (bf16); tcT = (p, c) via PE transp
  6. z[p, s-chunk] = pts^T @ x   (f32r, stationary = pts slices)
     e = exp(0.25 z) bf16 (no max subtraction; z/4 bounded ~ +-25)
  7. per s-tile: prop[s, c] = e-block^T @ tcT  (bf16), d[s] = e^T @ ones
     out[s, c] = prop * (1/d) + x_sc           (one DVE pass)
     output written (S, C); host transposes back to (C, T, H, W).
"""
import sys

sys.path.insert(0, "/opt/trn_rl_repo")

import numpy as np
import concourse.bass as bass
import concourse.mybir as mybir
import concourse.tile as tile
from concourse import bacc
from concourse.bass_utils import run_bass_kernel_spmd

F32 = mybir.dt.float32
F32R = mybir.dt.float32r
BF16 = mybir.dt.bfloat16
I16 = mybir.dt.int16
I32 = mybir.dt.int32
U32 = mybir.dt.uint32
AF = mybir.ActivationFunctionType
ALU = mybir.AluOpType
AX = mybir.AxisListType

B, C, T, H, W = 8, 512, 8, 28, 28
HW = H * W            # 784
S = T * HW            # 6272
R = 64
K = 4
Cq = 128
P = T * R             # 512
CC = C // 128         # 4
CE = 576              # gather row: 512 x + 2 coords + 62 pad (256B align)
NST = S // 128        # 49 s-tiles
NCH = 13              # s-chunks: 12 x 512 + 1 x 128

# affinity matmul dtype: f32r (fast) vs f32 (exact baseline fallback)
AFF_F32R = True

_CACHED = {}


def build_nc():
    nc = bacc.Bacc("TRN2", debug=False)

    X_CS = nc.dram_tensor("x_cs", [C, S], F32, kind="ExternalInput").ap()
    X_SC = nc.dram_tensor("x_sc", [S, CE], F32, kind="ExternalInput").ap()
    WRT = nc.dram_tensor("wrT", [C, R], F32, kind="ExternalInput").ap()
    WPT = nc.dram_tensor("wpT", [C, Cq], BF16, kind="ExternalInput").ap()
    WPC = nc.dram_tensor("wpc", [2, Cq], BF16, kind="ExternalInput").ap()
    WTT = nc.dram_tensor("wtT", [3, Cq, C], BF16, kind="ExternalInput").ap()
    B2 = nc.dram_tensor("b2", [Cq, 1], F32, kind="ExternalInput").ap()
    B3 = nc.dram_tensor("b3", [CC, 128], F32, kind="ExternalInput").ap()
    IDB = nc.dram_tensor("identbf", [128, 128], BF16, kind="ExternalInput").ap()
    IDF = nc.dram_tensor("identf", [128, 128], F32, kind="ExternalInput").ap()
    OUT = nc.dram_tensor("out_sc", [S, C], F32, kind="ExternalOutput").ap()

    TIDX = nc.dram_tensor("tidx_scr", [64], I16, kind="Internal").ap()
    GIDX = nc.dram_tensor("gidx_scr", [2048], I16, kind="Internal").ap()

    Xr = X_CS.rearrange("(cc p) s -> p cc s", p=128)
    XSCr = X_SC.rearrange("(n p) c -> p n c", p=128)
    OUTr = OUT.rearrange("(n p) c -> p n c", p=128)

    with tile.TileContext(nc) as tc:
        import contextlib
        ctx = contextlib.ExitStack()
        pers = ctx.enter_context(tc.tile_pool(name="pers", bufs=1))
        sb = ctx.enter_context(tc.tile_pool(name="sb", bufs=2))
        sb3 = ctx.enter_context(tc.tile_pool(name="sb3", bufs=3))
        ps = ctx.enter_context(tc.tile_pool(name="ps", bufs=3, space="PSUM"))
        pstp = ctx.enter_context(tc.tile_pool(name="pstp", bufs=2, space="PSUM"))
        pspr = ctx.enter_context(tc.tile_pool(name="pspr", bufs=2, space="PSUM"))
        psd = ctx.enter_context(tc.tile_pool(name="psd", bufs=1, space="PSUM"))

        # ---- persistent loads: frame 0 + weights first ----
        xc = pers.tile([128, CC, S], F32R, tag="xc")
        Xrr = Xr.bitcast(F32R)
        nc.sync.dma_start(out=xc[:, :, 0:392], in_=Xrr[:, :, 0:392])
        nc.sync.dma_start(out=xc[:, :, 392:HW], in_=Xrr[:, :, 392:HW])
        xcr = xc

        wrT_t = pers.tile([128, CC, R], F32R, tag="wrT")
        nc.sync.dma_start(out=wrT_t,
                          in_=WRT.rearrange("(cc p) r -> p cc r", p=128).bitcast(F32R))
        wpT_t = pers.tile([128, CC, Cq], BF16, tag="wpT")
        nc.sync.dma_start(out=wpT_t, in_=WPT.rearrange("(cc p) q -> p cc q", p=128))
        wpc_t = pers.tile([2, Cq], BF16, tag="wpc")
        nc.sync.dma_start(out=wpc_t, in_=WPC)
        wtT_t = pers.tile([128, 3, C], BF16, tag="wtT")
        nc.sync.dma_start(out=wtT_t, in_=WTT.rearrange("d p c -> p d c"))
        b2_t = pers.tile([128, 1], F32, tag="b2")
        nc.sync.dma_start(out=b2_t, in_=B2)
        b3_t = pers.tile([128, CC], F32, tag="b3")
        nc.sync.dma_start(out=b3_t, in_=B3.rearrange("cc p -> p cc"))
        idb_t = pers.tile([128, 128], BF16, tag="idb")
        nc.sync.dma_start(out=idb_t, in_=IDB)
        idf_t = pers.tile([128, 128], F32, tag="idf")
        nc.sync.dma_start(out=idf_t, in_=IDF)

        # rest of x: per-t-block DMAs so affinity can stream per t
        for tb in range(1, T):
            nc.sync.dma_start(out=xc[:, :, tb * HW:(tb + 1) * HW],
                              in_=Xrr[:, :, tb * HW:(tb + 1) * HW])

        # ---- phase 1: template + argmax ----
        tpl_sb = pers.tile([64, HW], F32, tag="tpl")
        for h in range(2):
            tp_ps = ps.tile([64, 392], F32, tag="mm")
            for cc in range(CC):
                nc.tensor.matmul(tp_ps,
                                 lhsT=wrT_t[:, cc, :],
                                 rhs=xcr[:, cc, h * 392:(h + 1) * 392],
                                 start=(cc == 0), stop=(cc == CC - 1))
            nc.scalar.activation(tpl_sb[:, h * 392:(h + 1) * 392], tp_ps, AF.Copy)
        tmx = pers.tile([64, 8], F32, tag="tmx")
        tmi = pers.tile([64, 8], U32, tag="tmi")
        nc.vector.max(out=tmx, in_=tpl_sb)
        nc.vector.max_index(out=tmi, in_max=tmx, in_values=tpl_sb)
        spt16 = pers.tile([64, 1], I16, tag="spt16")
        nc.vector.tensor_copy(spt16, tmi[:, 0:1])

        # stage spt idxs: 1 HWDGE write + 8 replica reads (wrapped 16p)
        nc.scalar.dma_start(out=TIDX.rearrange("(p one) -> p one", one=1),
                            in_=spt16)
        gidx64 = pers.tile([128, 4], I16, tag="gidx64")
        TIDXw = TIDX.rearrange("(c p) -> p c", p=16)
        for g in range(8):
            nc.scalar.dma_start(out=gidx64[16 * g:16 * (g + 1), :], in_=TIDXw)
        tresT = pers.tile([128, 1, CE], F32, tag="tresT")
        nc.gpsimd.dma_gather(out_ap=tresT, in_ap=X_SC, idxs_ap=gidx64,
                             num_idxs=64, num_idxs_reg=64, elem_size=CE)
        tres = pers.tile([128, CC, R], F32R, tag="tres")
        for cc in range(CC):
            tp = pstp.tile([128, 512], F32, tag="tp")
            nc.tensor.transpose(tp[:, 0:64],
                                tresT[0:64, 0, cc * 128:(cc + 1) * 128],
                                idf_t[0:64, 0:64])
            nc.scalar.activation(tres[:, cc, :], tp[:, 0:64], AF.Copy)

        # ---- phase 2: affinity + topk (per t; 64-partition tiles) ----
        # staging layout: gstage2[r, (k t)] so that dram j = 512k + 64t + r
        gstage2 = pers.tile([64, 32], I16, tag="gstage2")
        gs2_v = gstage2.rearrange("r (k t) -> r t k", t=8)
        tres_mm = tres if AFF_F32R else tres.bitcast(F32)
        xc_mm = xcr if AFF_F32R else xc.bitcast(F32)
        for t in range(T):
            aff_sb = sb.tile([64, HW], F32, tag="aff")
            for h in range(2):
                a_ps = ps.tile([64, 392], F32, tag="mm")
                for cc in range(CC):
                    nc.tensor.matmul(
                        a_ps,
                        lhsT=tres_mm[:, cc, :],
                        rhs=xc_mm[:, cc, t * HW + h * 392: t * HW + (h + 1) * 392],
                        start=(cc == 0), stop=(cc == CC - 1))
                nc.scalar.activation(aff_sb[:, h * 392:(h + 1) * 392], a_ps, AF.Copy)
            amx = sb.tile([64, 8], F32, tag="amx")
            ami = sb.tile([64, 8], U32, tag="ami")
            nc.vector.max(out=amx, in_=aff_sb)
            nc.vector.max_index(out=ami, in_max=amx, in_values=aff_sb)
            nc.vector.tensor_scalar(gs2_v[:, t, :], ami[:, 0:K],
                                    float(t * HW), None, op0=ALU.add)

        # stage traj idxs: 1 HWDGE write + 8 replica reads
        nc.scalar.dma_start(out=GIDX.rearrange("(k t r) -> r (k t)", r=64, t=T),
                            in_=gstage2)
        gidx2 = pers.tile([128, 128], I16, tag="gidx2")
        GIDXw = GIDX.rearrange("(c p) -> p c", p=16)
        for g in range(8):
            nc.scalar.dma_start(out=gidx2[16 * g:16 * (g + 1), :], in_=GIDXw)

        # ---- phase 3: traj gathers, fuse, points, conv ----
        fm_f32 = pers.tile([128, P], F32, tag="fmf")
        spts = pers.tile([128, 4, P], F32, tag="spts")
        gk = []
        for k in range(K):
            gk_t = sb.tile([128, 4, CE], F32, tag="gk")
            gk.append(gk_t)
            nc.gpsimd.dma_gather(out_ap=gk_t, in_ap=X_SC,
                                 idxs_ap=gidx2[:, k * 32:(k + 1) * 32],
                                 num_idxs=512, num_idxs_reg=512, elem_size=CE)
            trajk = sb.tile([128, CC, P], BF16, tag="trajk")
            for cc in range(CC):
                tp = pstp.tile([128, 512], F32, tag="tp")
                for jb in range(4):
                    nc.tensor.transpose(tp[:, jb * 128:(jb + 1) * 128],
                                        gk_t[:, jb, cc * 128:(cc + 1) * 128], idf_t)
                nc.scalar.activation(trajk[:, cc, :], tp, AF.Copy)
            # coords rows (gathered cols 512:514) -> (2, P)
            cd = psd.tile([2, 512], F32, tag="d")
            for jb in range(4):
                nc.tensor.transpose(cd[:, jb * 128:(jb + 1) * 128],
                                    gk_t[:, jb, 512:514], idf_t)
            coordk = sb.tile([2, P], BF16, tag="coordk")
            nc.vector.tensor_copy(coordk, cd)
            f_ps = ps.tile([128, P], F32, tag="mm")
            for cc in range(CC):
                nc.tensor.matmul(f_ps, lhsT=wpT_t[:, cc, :], rhs=trajk[:, cc, :],
                                 start=(cc == 0), stop=False)
            nc.tensor.matmul(f_ps, lhsT=wpc_t, rhs=coordk,
                             start=False, stop=True)
            if k == 0:
                nc.scalar.activation(fm_f32, f_ps, AF.Copy)
            else:
                nc.vector.tensor_tensor(out=fm_f32, in0=fm_f32, in1=f_ps, op=ALU.max)
            if k == 1:
                nc.vector.tensor_tensor(out=spts, in0=gk[0][:, :, 0:P],
                                        in1=gk[1][:, :, 0:P], op=ALU.add)
            elif k > 1:
                nc.vector.tensor_tensor(out=spts, in0=spts,
                                        in1=gk_t[:, :, 0:P], op=ALU.add)
        fm = pers.tile([128, P], BF16, tag="fm")
        nc.vector.tensor_scalar(fm, fm_f32, b2_t, None, op0=ALU.add)

        # points = sum_k traj_k (fp32); 1/4 folded into exp scale
        pts_f = pers.tile([128, CC, P], F32R, tag="ptsf")
        for cc in range(CC):
            tp = pstp.tile([128, 512], F32, tag="tp")
            for jb in range(4):
                nc.tensor.transpose(tp[:, jb * 128:(jb + 1) * 128],
                                    spts[:, jb, cc * 128:(cc + 1) * 128], idf_t)
            nc.scalar.activation(pts_f[:, cc, :], tp, AF.Copy)
        ptsr = pts_f

        # conv over t (3 taps) + bias3 + relu -> tc bf16
        tc_bf = pers.tile([128, CC, P], BF16, tag="tcbf")
        for ct in range(CC):
            c_ps = ps.tile([128, P], F32, tag="mm")
            cs = slice(ct * 128, (ct + 1) * 128)
            nc.tensor.matmul(c_ps, lhsT=wtT_t[:, 1, cs], rhs=fm,
                             start=True, stop=False)
            nc.tensor.matmul(c_ps[:, R:P], lhsT=wtT_t[:, 0, cs], rhs=fm[:, 0:P - R],
                             start=False, stop=False)
            nc.tensor.matmul(c_ps[:, 0:P - R], lhsT=wtT_t[:, 2, cs], rhs=fm[:, R:P],
                             start=False, stop=True)
            nc.scalar.activation(tc_bf[:, ct, :], c_ps, AF.Relu,
                                 bias=b3_t[:, ct:ct + 1])
        # tcT: (p, c) layout for prop matmuls
        tcT = pers.tile([128, CC, C], BF16, tag="tcT")
        for pb in range(4):
            tp2 = pstp.tile([128, 512], BF16, tag="tp")
            for cc in range(CC):
                nc.tensor.transpose(tp2[:, cc * 128:(cc + 1) * 128],
                                    tc_bf[:, cc, pb * 128:(pb + 1) * 128], idb_t)
            nc.vector.tensor_copy(tcT[:, pb, :], tp2)

        # ---- phase 4: z (s,p) -> softmax -> prop -> out, per s-tile ----
        # z per s-tile: lhsT = x slices, rhs = pts (f32r).  exp with
        # per-tile max bias + accumulated denominator; e transposed via PE
        # (bf16) and fed back as prop lhsT; normalize + residual fused in
        # one DVE pass at the output.
        for i in range(NCH):
            cw = min(P, S - i * P)
            nt = cw // 128
            xres = sb.tile([128, 4, C], F32, tag="xres")
            nc.sync.dma_start(out=xres[:, 0:nt, :],
                              in_=XSCr[:, 4 * i:4 * i + nt, 0:C])
            for t in range(nt):
                st = 4 * i + t
                z_ps = ps.tile([128, P], F32, tag="mm")
                for cc in range(CC):
                    nc.tensor.matmul(z_ps,
                                     lhsT=xcr[:, cc, st * 128:(st + 1) * 128],
                                     rhs=ptsr[:, cc, :],
                                     start=(cc == 0), stop=(cc == CC - 1))
                nm = sb3.tile([128, 1], F32, tag="nm")
                nc.vector.tensor_reduce(nm, z_ps, axis=AX.X, op=ALU.max,
                                        negate=True)
                nm4 = sb3.tile([128, 1], F32, tag="nm4")
                nc.vector.tensor_scalar(nm4, nm, 0.25, None, op0=ALU.mult)
                e_sb = sb3.tile([128, P], BF16, tag="esb")
                dsum = sb3.tile([128, 1], F32, tag="dsum")
                nc.scalar.activation(e_sb, z_ps, AF.Exp, bias=nm4, scale=0.25,
                                     accum_out=dsum)
                eT_ps = pstp.tile([128, 512], BF16, tag="tp")
                for pb in range(4):
                    nc.tensor.transpose(eT_ps[:, pb * 128:(pb + 1) * 128],
                                        e_sb[:, pb * 128:(pb + 1) * 128], idb_t)
                eT = sb3.tile([128, 512], BF16, tag="eT")
                nc.scalar.activation(eT, eT_ps, AF.Copy)
                pr = pspr.tile([128, C], F32, tag="pr")
                for pb in range(4):
                    nc.tensor.matmul(pr, lhsT=eT[:, pb * 128:(pb + 1) * 128],
                                     rhs=tcT[:, pb, :],
                                     start=(pb == 0), stop=(pb == CC - 1))
                rd = sb3.tile([128, 1], F32, tag="rd")
                nc.vector.reciprocal(rd, dsum)
                osb = sb3.tile([128, C], F32, tag="osb")
                nc.vector.scalar_tensor_tensor(
                    osb, in0=pr, scalar=rd, in1=xres[:, t, :],
                    op0=ALU.mult, op1=ALU.add)
                nc.sync.dma_start(out=OUTr[:, st, :], in_=osb)
        ctx.close()
    nc.compile()
    return nc


def _host_prep(inputs):
    eps = 1e-5
    f32 = np.float32
    import ml_dtypes
    bf16 = ml_dtypes.bfloat16
    x = np.asarray(inputs["input"], f32)                       # (B,C,T,H,W)
    s1 = np.asarray(inputs["bn1_gamma"]) / np.sqrt(np.asarray(inputs["bn1_var"]) + eps)
    wrT = (np.asarray(inputs["w_reduce"], f32) * s1[:, None]).T.astype(f32)
    s2 = np.asarray(inputs["bn2_gamma"]) / np.sqrt(np.asarray(inputs["bn2_var"]) + eps)
    wp = np.asarray(inputs["w_proj"], f32) * s2[:, None]       # (Cq, C+2)
    b2 = (np.asarray(inputs["bn2_beta"])
          - np.asarray(inputs["bn2_mean"]) * s2).astype(f32)
    s3 = np.asarray(inputs["bn3_gamma"]) / np.sqrt(np.asarray(inputs["bn3_var"]) + eps)
    wt = np.asarray(inputs["w_t"], f32)[:, :, :, 0] * s3[:, None, None]  # (C,Cq,3)
    b3 = (np.asarray(inputs["bn3_beta"])
          - np.asarray(inputs["bn3_mean"]) * s3).astype(f32)
    common = {
        "wrT": np.ascontiguousarray(wrT),
        "wpT": np.ascontiguousarray(wp[:, :C].T.astype(bf16)),
        "wpc": np.ascontiguousarray(wp[:, C:].T.astype(bf16)),
        "wtT": np.ascontiguousarray(np.transpose(wt, (2, 1, 0)).astype(bf16)),
        "b2": b2.reshape(Cq, 1),
        "b3": b3.reshape(CC, 128),
        "identbf": np.eye(128, dtype=bf16),
        "identf": np.eye(128, dtype=f32),
    }
    x_cs = x.reshape(B, C, S)
    # augmented (S, CE) per-sample: x^T | row/H | col/W | zero pad
    hw_idx = np.arange(HW, dtype=f32)
    rowv = np.tile((hw_idx // W) / H, T)                       # (S,)
    colv = np.tile((hw_idx % W) / W, T)
    in_maps = []
    for b in range(B):
        m = dict(common)
        m["x_cs"] = np.ascontiguousarray(x_cs[b])
        aug = np.zeros((S, CE), f32)
        aug[:, :C] = x_cs[b].T
        aug[:, C] = rowv
        aug[:, C + 1] = colv
        m["x_sc"] = aug
        in_maps.append(m)
    return in_maps


def kernel(**inputs) -> np.ndarray:
    if "nc" not in _CACHED:
        _CACHED["nc"] = build_nc()
    nc = _CACHED["nc"]
    in_maps = _host_prep(inputs)
    res = run_bass_kernel_spmd(nc, in_maps, list(range(B)))
    out = np.stack([res.results[b]["out_sc"].T for b in range(B)], axis=0)
    return out.reshape(B, C, T, H, W).astype(np.float32)


# revision 13
# speedup vs baseline: 1.5679x; 1.1089x over previous
"""Trainium2 Bass kernel for nn_CorrTrajBlock (sparse_attention).

Data-parallel over batch B=8 across 8 NeuronCores; one sample per core.

Per-core pipeline (C=512, T=8, H=W=28, HW=784, S=T*HW=6272, R=64, K=4,
Cq=128, P=T*R=512):
  1. template_p = w_reduce_eff @ x[:, 0]        (f32r matmul, 64x784)
     spt_inds   = argmax over HW                (DVE max/max_index)
  2. template_resample gather (64 rows of x_sc_aug), PE-transpose
  3. affinity = template_resample^T @ x_flat    (f32r matmul, 64x6272)
     topk4 per (r, t) over HW                   (DVE max/max_index)
  4. traj gather (2048 rows of x_sc_aug, coords baked in cols 512:514),
     PE-transpose to (c, ktr); points = sum_k traj (DVE adds + PE transp)
  5. fuse = w_proj_eff @ [traj; coords] (bf16) -> max over k -> +bias2
     tc = relu(conv_t(fuse) + bias3) (bf16); tcT = (p, c) via PE transp
  6. z[p, s-chunk] = pts^T @ x   (f32r, stationary = pts slices)
     e = exp(0.25 z) bf16 (no max subtraction; z/4 bounded ~ +-25)
  7. per s-tile: prop[s, c] = e-block^T @ tcT  (bf16), d[s] = e^T @ ones
     out[s, c] = prop * (1/d) + x_sc           (one DVE pass)
     output written (S, C); host transposes back to (C, T, H, W).
"""
import sys

sys.path.insert(0, "/opt/trn_rl_repo")

import numpy as np
import concourse.bass as bass
import concourse.mybir as mybir
import concourse.tile as tile
from concourse import bacc
from concourse.bass_utils import run_bass_kernel_spmd

F32 = mybir.dt.float32
F32R = mybir.dt.float32r
BF16 = mybir.dt.bfloat16
I16 = mybir.dt.int16
I32 = mybir.dt.int32
U32 = mybir.dt.uint32
AF = mybir.ActivationFunctionType
ALU = mybir.AluOpType
AX = mybir.AxisListType

B, C, T, H, W = 8, 512, 8, 28, 28
HW = H * W            # 784
S = T * HW            # 6272
R = 64
K = 4
Cq = 128
P = T * R             # 512
CC = C // 128         # 4
CE = 576              # gather row: 512 x + 2 coords + 62 pad (256B align)
NST = S // 128        # 49 s-tiles
NCH = 13              # s-chunks: 12 x 512 + 1 x 128

# affinity matmul dtype: f32r (fast) vs f32 (exact baseline fallback)
AFF_F32R = True

_CACHED = {}


def build_nc():
    nc = bacc.Bacc("TRN2", debug=False)

    X_CS = nc.dram_tensor("x_cs", [C, S], F32, kind="ExternalInput").ap()
    X_SC = nc.dram_tensor("x_sc", [S, CE], F32, kind="ExternalInput").ap()
    WRT = nc.dram_tensor("wrT", [C, R], F32, kind="ExternalInput").ap()
    WPT = nc.dram_tensor("wpT", [C, Cq], BF16, kind="ExternalInput").ap()
    WPC = nc.dram_tensor("wpc", [2, Cq], BF16, kind="ExternalInput").ap()
    WTT = nc.dram_tensor("wtT", [3, Cq, C], BF16, kind="ExternalInput").ap()
    B2 = nc.dram_tensor("b2", [Cq, 1], F32, kind="ExternalInput").ap()
    B3 = nc.dram_tensor("b3", [CC, 128], F32, kind="ExternalInput").ap()
    IDB = nc.dram_tensor("identbf", [128, 128], BF16, kind="ExternalInput").ap()
    IDF = nc.dram_tensor("identf", [128, 128], F32, kind="ExternalInput").ap()
    OUT = nc.dram_tensor("out_sc", [S, C], F32, kind="ExternalOutput").ap()

    TIDX = nc.dram_tensor("tidx_scr", [64], I16, kind="Internal").ap()

    Xr = X_CS.rearrange("(cc p) s -> p cc s", p=128)
    XSCr = X_SC.rearrange("(n p) c -> p n c", p=128)
    OUTr = OUT.rearrange("(n p) c -> p n c", p=128)

    with tile.TileContext(nc) as tc:
        import contextlib
        ctx = contextlib.ExitStack()
        pers = ctx.enter_context(tc.tile_pool(name="pers", bufs=1))
        sb = ctx.enter_context(tc.tile_pool(name="sb", bufs=2))
        sb3 = ctx.enter_context(tc.tile_pool(name="sb3", bufs=3))
        ps = ctx.enter_context(tc.tile_pool(name="ps", bufs=3, space="PSUM"))
        pstp = ctx.enter_context(tc.tile_pool(name="pstp", bufs=2, space="PSUM"))
        pspr = ctx.enter_context(tc.tile_pool(name="pspr", bufs=3, space="PSUM"))

        # ---- persistent loads: frame 0 + weights first ----
        xc = pers.tile([128, CC, S], F32R, tag="xc")
        Xrr = Xr.bitcast(F32R)
        nc.sync.dma_start(out=xc[:, :, 0:392], in_=Xrr[:, :, 0:392])
        nc.sync.dma_start(out=xc[:, :, 392:HW], in_=Xrr[:, :, 392:HW])
        xcr = xc

        wrT_t = pers.tile([128, CC, R], F32R, tag="wrT")
        nc.sync.dma_start(out=wrT_t,
                          in_=WRT.rearrange("(cc p) r -> p cc r", p=128).bitcast(F32R))
        wpT_t = pers.tile([128, CC, Cq], BF16, tag="wpT")
        nc.sync.dma_start(out=wpT_t, in_=WPT.rearrange("(cc p) q -> p cc q", p=128))
        wpc_t = pers.tile([2, Cq], BF16, tag="wpc")
        nc.sync.dma_start(out=wpc_t, in_=WPC)
        wtT_t = pers.tile([128, 3, C], BF16, tag="wtT")
        nc.sync.dma_start(out=wtT_t, in_=WTT.rearrange("d p c -> p d c"))
        b2_t = pers.tile([128, 1], F32, tag="b2")
        nc.sync.dma_start(out=b2_t, in_=B2)
        b3_t = pers.tile([128, CC], F32, tag="b3")
        nc.sync.dma_start(out=b3_t, in_=B3.rearrange("cc p -> p cc"))
        idb_t = pers.tile([128, 128], BF16, tag="idb")
        nc.sync.dma_start(out=idb_t, in_=IDB)
        idf_t = pers.tile([128, 128], F32, tag="idf")
        nc.sync.dma_start(out=idf_t, in_=IDF)

        # ---- phase 1: template + argmax ----
        tpl_sb = pers.tile([64, HW], F32, tag="tpl")
        for h in range(2):
            tp_ps = ps.tile([64, 392], F32, tag="mm")
            for cc in range(CC):
                nc.tensor.matmul(tp_ps,
                                 lhsT=wrT_t[:, cc, :],
                                 rhs=xcr[:, cc, h * 392:(h + 1) * 392],
                                 start=(cc == 0), stop=(cc == CC - 1))
            nc.scalar.activation(tpl_sb[:, h * 392:(h + 1) * 392], tp_ps, AF.Copy)
        tmx = pers.tile([64, 8], F32, tag="tmx")
        tmi = pers.tile([64, 8], U32, tag="tmi")
        nc.vector.max(out=tmx, in_=tpl_sb)
        nc.vector.max_index(out=tmi, in_max=tmx, in_values=tpl_sb)
        spt16 = pers.tile([64, 1], I16, tag="spt16")
        nc.vector.tensor_copy(spt16, tmi[:, 0:1])

        # stage spt idxs: 1 HWDGE write + 8 replica reads (wrapped 16p)
        nc.scalar.dma_start(out=TIDX.rearrange("(p one) -> p one", one=1),
                            in_=spt16)
        gidx64 = pers.tile([128, 4], I16, tag="gidx64")
        TIDXw = TIDX.rearrange("(c p) -> p c", p=16)
        for g in range(8):
            nc.scalar.dma_start(out=gidx64[16 * g:16 * (g + 1), :], in_=TIDXw)
        tresT = pers.tile([128, 1, CE], F32, tag="tresT")
        gthr_inst = nc.gpsimd.dma_gather(
            out_ap=tresT, in_ap=X_SC, idxs_ap=gidx64,
            num_idxs=64, num_idxs_reg=64, elem_size=CE)
        # rest of x loads AFTER the tres staging so its small DMAs are not
        # stuck behind megabytes of bulk traffic in the shared SDMA rings
        from concourse.tile_rust import add_dep_helper
        first_tb = None
        for tb in range(1, T):
            for hh in range(2):
                i0 = tb * HW + hh * 392
                dma = nc.sync.dma_start(out=xc[:, :, i0:i0 + 392],
                                        in_=Xrr[:, :, i0:i0 + 392])
                if first_tb is None:
                    first_tb = dma
                    try:
                        add_dep_helper(dma.ins, gthr_inst.ins, sync=True,
                                       reason="bulk x loads after idx staging")
                    except AttributeError:
                        add_dep_helper(dma, gthr_inst, sync=True,
                                       reason="bulk x loads after idx staging")
        tres = pers.tile([128, CC, R], F32R, tag="tres")
        for cc in range(CC):
            tp = pstp.tile([128, 512], F32, tag="tp")
            nc.tensor.transpose(tp[:, 0:64],
                                tresT[0:64, 0, cc * 128:(cc + 1) * 128],
                                idf_t[0:64, 0:64])
            nc.scalar.activation(tres[:, cc, :], tp[:, 0:64], AF.Copy)

        # ---- phase 2: affinity + topk (per t; 64-partition tiles) ----
        # staging layout: gstage2[r, (k t)] so that dram j = 512k + 64t + r
        gstage2 = pers.tile([64, 32], F32, tag="gstage2")
        gs2_v = gstage2.rearrange("r (k t) -> r t k", t=8)
        tres_mm = tres if AFF_F32R else tres.bitcast(F32)
        xc_mm = xcr if AFF_F32R else xc.bitcast(F32)
        for t in range(T):
            aff_sb = sb.tile([64, HW], F32, tag="aff")
            for h in range(2):
                a_ps = ps.tile([64, 392], F32, tag="mm")
                for cc in range(CC):
                    nc.tensor.matmul(
                        a_ps,
                        lhsT=tres_mm[:, cc, :],
                        rhs=xc_mm[:, cc, t * HW + h * 392: t * HW + (h + 1) * 392],
                        start=(cc == 0), stop=(cc == CC - 1))
                nc.scalar.activation(aff_sb[:, h * 392:(h + 1) * 392], a_ps, AF.Copy)
            amx = sb.tile([64, 8], F32, tag="amx")
            ami = sb.tile([64, 8], U32, tag="ami")
            nc.vector.max(out=amx, in_=aff_sb)
            nc.vector.max_index(out=ami, in_max=amx, in_values=aff_sb)
            nc.vector.tensor_scalar(gs2_v[:, t, :], ami[:, 0:K],
                                    float(t * HW), None, op0=ALU.add)

        # build wrapped idx layout on-chip: gstage2[r=16rh+p16, q=8k+t]
        # -> w16[p16, 4q+rh] via PE int16 transposes, then replicate 8x.
        t1_ps = pstp.tile([32, 64], F32, tag="tp")
        nc.tensor.transpose(t1_ps, gstage2, idf_t[0:64, 0:64])
        t1 = pers.tile([32, 64], F32, tag="t1")
        nc.vector.tensor_copy(t1, t1_ps)
        w16 = pers.tile([16, 128], I16, tag="w16")
        w16v = w16.rearrange("p (q rh) -> p q rh", rh=4)
        for rh in range(4):
            wr_ps = pstp.tile([16, 32], F32, tag="tp")
            nc.tensor.transpose(wr_ps, t1[:, 16 * rh:16 * (rh + 1)],
                                idf_t[0:32, 0:32])
            nc.vector.tensor_copy(w16v[:, :, rh], wr_ps)
        gidx2 = pers.tile([128, 128], I16, tag="gidx2")
        for g in range(8):
            nc.scalar.dma_start(out=gidx2[16 * g:16 * (g + 1), :], in_=w16)

        # ---- phase 3: traj gathers, fuse, points, conv ----
        fm_f32 = pers.tile([128, P], F32, tag="fmf")
        spts = pers.tile([128, 4, P], F32, tag="spts")
        gk = []
        for k in range(K):
            gk_t = sb.tile([128, 4, CE], F32, tag="gk")
            gk.append(gk_t)
            nc.gpsimd.dma_gather(out_ap=gk_t, in_ap=X_SC,
                                 idxs_ap=gidx2[:, k * 32:(k + 1) * 32],
                                 num_idxs=512, num_idxs_reg=512, elem_size=CE)
            trajk = sb.tile([128, CC, P], BF16, tag="trajk")
            for cc in range(CC):
                tp = pstp.tile([128, 512], F32, tag="tp")
                for jb in range(4):
                    nc.tensor.transpose(tp[:, jb * 128:(jb + 1) * 128],
                                        gk_t[:, jb, cc * 128:(cc + 1) * 128], idf_t)
                nc.scalar.activation(trajk[:, cc, :], tp, AF.Copy)
            # coords rows (gathered cols 512:514) -> (2, P)
            cd = pstp.tile([2, 512], F32, tag="tp")
            for jb in range(4):
                nc.tensor.transpose(cd[:, jb * 128:(jb + 1) * 128],
                                    gk_t[:, jb, 512:514], idf_t)
            coordk = sb.tile([2, P], BF16, tag="coordk")
            nc.vector.tensor_copy(coordk, cd)
            f_ps = ps.tile([128, P], F32, tag="mm")
            for cc in range(CC):
                nc.tensor.matmul(f_ps, lhsT=wpT_t[:, cc, :], rhs=trajk[:, cc, :],
                                 start=(cc == 0), stop=False)
            nc.tensor.matmul(f_ps, lhsT=wpc_t, rhs=coordk,
                             start=False, stop=True)
            if k == 0:
                nc.scalar.activation(fm_f32, f_ps, AF.Copy)
            else:
                nc.vector.tensor_tensor(out=fm_f32, in0=fm_f32, in1=f_ps, op=ALU.max)
            if k == 1:
                nc.vector.tensor_tensor(out=spts, in0=gk[0][:, :, 0:P],
                                        in1=gk[1][:, :, 0:P], op=ALU.add)
            elif k > 1:
                nc.vector.tensor_tensor(out=spts, in0=spts,
                                        in1=gk_t[:, :, 0:P], op=ALU.add)
        fm = pers.tile([128, P], BF16, tag="fm")
        nc.vector.tensor_scalar(fm, fm_f32, b2_t, None, op0=ALU.add)

        # points = sum_k traj_k (fp32); 1/4 folded into exp scale
        pts_f = pers.tile([128, CC, P], F32R, tag="ptsf")
        for cc in range(CC):
            tp = pstp.tile([128, 512], F32, tag="tp")
            for jb in range(4):
                nc.tensor.transpose(tp[:, jb * 128:(jb + 1) * 128],
                                    spts[:, jb, cc * 128:(cc + 1) * 128], idf_t)
            nc.scalar.activation(pts_f[:, cc, :], tp, AF.Copy)
        ptsr = pts_f

        # conv over t (3 taps) + bias3 + relu -> tc bf16
        tc_bf = pers.tile([128, CC, P], BF16, tag="tcbf")
        for ct in range(CC):
            c_ps = ps.tile([128, P], F32, tag="mm")
            cs = slice(ct * 128, (ct + 1) * 128)
            nc.tensor.matmul(c_ps, lhsT=wtT_t[:, 1, cs], rhs=fm,
                             start=True, stop=False)
            nc.tensor.matmul(c_ps[:, R:P], lhsT=wtT_t[:, 0, cs], rhs=fm[:, 0:P - R],
                             start=False, stop=False)
            nc.tensor.matmul(c_ps[:, 0:P - R], lhsT=wtT_t[:, 2, cs], rhs=fm[:, R:P],
                             start=False, stop=True)
            nc.scalar.activation(tc_bf[:, ct, :], c_ps, AF.Relu,
                                 bias=b3_t[:, ct:ct + 1])
        # tcT: (p, c) layout for prop matmuls
        tcT = pers.tile([128, CC, C], BF16, tag="tcT")
        for pb in range(4):
            tp2 = pstp.tile([128, 512], BF16, tag="tp")
            for cc in range(CC):
                nc.tensor.transpose(tp2[:, cc * 128:(cc + 1) * 128],
                                    tc_bf[:, cc, pb * 128:(pb + 1) * 128], idb_t)
            nc.vector.tensor_copy(tcT[:, pb, :], tp2)

        # ---- phase 4: z (s,p) -> softmax -> prop -> out, per s-tile ----
        # z per s-tile: lhsT = x slices, rhs = pts (f32r).  exp with
        # per-tile max bias + accumulated denominator; e transposed via PE
        # (bf16) and fed back as prop lhsT; normalize + residual fused in
        # one DVE pass at the output.
        for i in range(NCH):
            cw = min(P, S - i * P)
            nt = cw // 128
            xres = sb.tile([128, 4, C], F32, tag="xres")
            nc.sync.dma_start(out=xres[:, 0:nt, :],
                              in_=XSCr[:, 4 * i:4 * i + nt, 0:C])
            for t in range(nt):
                st = 4 * i + t
                z_ps = ps.tile([128, P], F32, tag="mm")
                for cc in range(CC):
                    nc.tensor.matmul(z_ps,
                                     lhsT=xcr[:, cc, st * 128:(st + 1) * 128],
                                     rhs=ptsr[:, cc, :],
                                     start=(cc == 0), stop=(cc == CC - 1))
                nm = sb3.tile([128, 1], F32, tag="nm")
                nc.vector.tensor_reduce(nm, z_ps, axis=AX.X, op=ALU.max,
                                        negate=True)
                nm4 = sb3.tile([128, 1], F32, tag="nm4")
                nc.vector.tensor_scalar(nm4, nm, 0.25, None, op0=ALU.mult)
                e_sb = sb3.tile([128, P], BF16, tag="esb")
                dsum = sb3.tile([128, 1], F32, tag="dsum")
                nc.scalar.activation(e_sb, z_ps, AF.Exp, bias=nm4, scale=0.25,
                                     accum_out=dsum)
                eT_ps = pstp.tile([128, 512], BF16, tag="tp")
                for pb in range(4):
                    nc.tensor.transpose(eT_ps[:, pb * 128:(pb + 1) * 128],
                                        e_sb[:, pb * 128:(pb + 1) * 128], idb_t)
                eT = sb3.tile([128, 512], BF16, tag="eT")
                nc.scalar.activation(eT, eT_ps, AF.Copy)
                pr = pspr.tile([128, C], F32, tag="pr")
                for pb in range(4):
                    nc.tensor.matmul(pr, lhsT=eT[:, pb * 128:(pb + 1) * 128],
                                     rhs=tcT[:, pb, :],
                                     start=(pb == 0), stop=(pb == CC - 1))
                rd = sb3.tile([128, 1], F32, tag="rd")
                nc.vector.reciprocal(rd, dsum)
                osb = sb3.tile([128, C], F32, tag="osb")
                nc.vector.scalar_tensor_tensor(
                    osb, in0=pr, scalar=rd, in1=xres[:, t, :],
                    op0=ALU.mult, op1=ALU.add)
                nc.sync.dma_start(out=OUTr[:, st, :], in_=osb)
        ctx.close()
    nc.compile()
    return nc


def _host_prep(inputs):
    eps = 1e-5
    f32 = np.float32
    import ml_dtypes
    bf16 = ml_dtypes.bfloat16
    x = np.asarray(inputs["input"], f32)                       # (B,C,T,H,W)
    s1 = np.asarray(inputs["bn1_gamma"]) / np.sqrt(np.asarray(inputs["bn1_var"]) + eps)
    wrT = (np.asarray(inputs["w_reduce"], f32) * s1[:, None]).T.astype(f32)
    s2 = np.asarray(inputs["bn2_gamma"]) / np.sqrt(np.asarray(inputs["bn2_var"]) + eps)
    wp = np.asarray(inputs["w_proj"], f32) * s2[:, None]       # (Cq, C+2)
    b2 = (np.asarray(inputs["bn2_beta"])
          - np.asarray(inputs["bn2_mean"]) * s2).astype(f32)
    s3 = np.asarray(inputs["bn3_gamma"]) / np.sqrt(np.asarray(inputs["bn3_var"]) + eps)
    wt = np.asarray(inputs["w_t"], f32)[:, :, :, 0] * s3[:, None, None]  # (C,Cq,3)
    b3 = (np.asarray(inputs["bn3_beta"])
          - np.asarray(inputs["bn3_mean"]) * s3).astype(f32)
    common = {
        "wrT": np.ascontiguousarray(wrT),
        "wpT": np.ascontiguousarray(wp[:, :C].T.astype(bf16)),
        "wpc": np.ascontiguousarray(wp[:, C:].T.astype(bf16)),
        "wtT": np.ascontiguousarray(np.transpose(wt, (2, 1, 0)).astype(bf16)),
        "b2": b2.reshape(Cq, 1),
        "b3": b3.reshape(CC, 128),
        "identbf": np.eye(128, dtype=bf16),
        "identf": np.eye(128, dtype=f32),
    }
    x_cs = x.reshape(B, C, S)
    # augmented (S, CE) per-sample: x^T | row/H | col/W | zero pad
    hw_idx = np.arange(HW, dtype=f32)
    rowv = np.tile((hw_idx // W) / H, T)                       # (S,)
    colv = np.tile((hw_idx % W) / W, T)
    in_maps = []
    for b in range(B):
        m = dict(common)
        m["x_cs"] = np.ascontiguousarray(x_cs[b])
        aug = np.zeros((S, CE), f32)
        aug[:, :C] = x_cs[b].T
        aug[:, C] = rowv
        aug[:, C + 1] = colv
        m["x_sc"] = aug
        in_maps.append(m)
    return in_maps


def kernel(**inputs) -> np.ndarray:
    if "nc" not in _CACHED:
        _CACHED["nc"] = build_nc()
    nc = _CACHED["nc"]
    in_maps = _host_prep(inputs)
    res = run_bass_kernel_spmd(nc, in_maps, list(range(B)))
    out = np.stack([res.results[b]["out_sc"].T for b in range(B)], axis=0)
    return out.reshape(B, C, T, H, W).astype(np.float32)


# revision 14
# speedup vs baseline: 1.6985x; 1.0833x over previous
"""Trainium2 Bass kernel for nn_CorrTrajBlock (sparse_attention).

Data-parallel over batch B=8 across 8 NeuronCores; one sample per core.

Per-core pipeline (C=512, T=8, H=W=28, HW=784, S=T*HW=6272, R=64, K=4,
Cq=128, P=T*R=512):
  1. template_p = w_reduce_eff @ x[:, 0]        (f32r matmul, 64x784)
     spt_inds   = argmax over HW                (DVE max/max_index)
  2. template_resample gather (64 rows of x_sc_aug), PE-transpose
  3. affinity = template_resample^T @ x_flat    (f32r matmul, 64x6272)
     topk4 per (r, t) over HW                   (DVE max/max_index)
  4. traj gather (2048 rows of x_sc_aug, coords baked in cols 512:514),
     PE-transpose to (c, ktr); points = sum_k traj (DVE adds + PE transp)
  5. fuse = w_proj_eff @ [traj; coords] (bf16) -> max over k -> +bias2
     tc = relu(conv_t(fuse) + bias3) (bf16); tcT = (p, c) via PE transp
  6. z[p, s-chunk] = pts^T @ x   (f32r, stationary = pts slices)
     e = exp(0.25 z) bf16 (no max subtraction; z/4 bounded ~ +-25)
  7. per s-tile: prop[s, c] = e-block^T @ tcT  (bf16), d[s] = e^T @ ones
     out[s, c] = prop * (1/d) + x_sc           (one DVE pass)
     output written (S, C); host transposes back to (C, T, H, W).
"""
import sys

sys.path.insert(0, "/opt/trn_rl_repo")

import numpy as np
import concourse.bass as bass
import concourse.mybir as mybir
import concourse.tile as tile
from concourse import bacc
from concourse.bass_utils import run_bass_kernel_spmd

F32 = mybir.dt.float32
F32R = mybir.dt.float32r
BF16 = mybir.dt.bfloat16
I16 = mybir.dt.int16
I32 = mybir.dt.int32
U32 = mybir.dt.uint32
AF = mybir.ActivationFunctionType
ALU = mybir.AluOpType
AX = mybir.AxisListType

B, C, T, H, W = 8, 512, 8, 28, 28
HW = H * W            # 784
S = T * HW            # 6272
R = 64
K = 4
Cq = 128
P = T * R             # 512
CC = C // 128         # 4
CE = 576              # gather row: 512 x + 2 coords + 62 pad (256B align)
NST = S // 128        # 49 s-tiles
NCH = 13              # s-chunks: 12 x 512 + 1 x 128

# affinity matmul dtype: f32r (fast) vs f32 (exact baseline fallback)
AFF_F32R = True

_CACHED = {}


def build_nc():
    nc = bacc.Bacc("TRN2", debug=False)

    X_CS = nc.dram_tensor("x_cs", [C, S], F32, kind="ExternalInput").ap()
    X_SC = nc.dram_tensor("x_sc", [S, CE], F32, kind="ExternalInput").ap()
    WRT = nc.dram_tensor("wrT", [C, R], F32, kind="ExternalInput").ap()
    WPT = nc.dram_tensor("wpT", [C, Cq], BF16, kind="ExternalInput").ap()
    WPC = nc.dram_tensor("wpc", [2, Cq], BF16, kind="ExternalInput").ap()
    WTT = nc.dram_tensor("wtT", [3, Cq, C], BF16, kind="ExternalInput").ap()
    B2 = nc.dram_tensor("b2", [Cq, 1], F32, kind="ExternalInput").ap()
    B3 = nc.dram_tensor("b3", [CC, 128], F32, kind="ExternalInput").ap()
    IDB = nc.dram_tensor("identbf", [128, 128], BF16, kind="ExternalInput").ap()
    IDF = nc.dram_tensor("identf", [128, 128], F32, kind="ExternalInput").ap()
    OUT = nc.dram_tensor("out_sc", [S, C], F32, kind="ExternalOutput").ap()


    Xr = X_CS.rearrange("(cc p) s -> p cc s", p=128)
    XSCr = X_SC.rearrange("(n p) c -> p n c", p=128)
    OUTr = OUT.rearrange("(n p) c -> p n c", p=128)

    with tile.TileContext(nc) as tc:
        import contextlib
        ctx = contextlib.ExitStack()
        pers = ctx.enter_context(tc.tile_pool(name="pers", bufs=1))
        sb = ctx.enter_context(tc.tile_pool(name="sb", bufs=2))
        sb3 = ctx.enter_context(tc.tile_pool(name="sb3", bufs=3))
        ps = ctx.enter_context(tc.tile_pool(name="ps", bufs=3, space="PSUM"))
        pstp = ctx.enter_context(tc.tile_pool(name="pstp", bufs=2, space="PSUM"))
        pspr = ctx.enter_context(tc.tile_pool(name="pspr", bufs=3, space="PSUM"))

        # ---- persistent loads: frame 0 + weights first ----
        wrT_t = pers.tile([128, CC, R], F32R, tag="wrT")
        nc.sync.dma_start(out=wrT_t,
                          in_=WRT.rearrange("(cc p) r -> p cc r", p=128).bitcast(F32R))
        xc = pers.tile([128, CC, S], F32R, tag="xc")
        Xrr = Xr.bitcast(F32R)
        nc.sync.dma_start(out=xc[:, :, 0:392], in_=Xrr[:, :, 0:392])
        nc.sync.dma_start(out=xc[:, :, 392:HW], in_=Xrr[:, :, 392:HW])
        xcr = xc
        wpT_t = pers.tile([128, CC, Cq], BF16, tag="wpT")
        nc.sync.dma_start(out=wpT_t, in_=WPT.rearrange("(cc p) q -> p cc q", p=128))
        wpc_t = pers.tile([2, Cq], BF16, tag="wpc")
        nc.sync.dma_start(out=wpc_t, in_=WPC)
        wtT_t = pers.tile([128, 3, C], BF16, tag="wtT")
        nc.sync.dma_start(out=wtT_t, in_=WTT.rearrange("d p c -> p d c"))
        b2_t = pers.tile([128, 1], F32, tag="b2")
        nc.sync.dma_start(out=b2_t, in_=B2)
        b3_t = pers.tile([128, CC], F32, tag="b3")
        nc.sync.dma_start(out=b3_t, in_=B3.rearrange("cc p -> p cc"))
        idb_t = pers.tile([128, 128], BF16, tag="idb")
        nc.sync.dma_start(out=idb_t, in_=IDB)
        idf_t = pers.tile([128, 128], F32, tag="idf")
        nc.sync.dma_start(out=idf_t, in_=IDF)

        # ---- phase 1: template + argmax ----
        tpl_sb = pers.tile([64, HW], F32, tag="tpl")
        for h in range(2):
            tp_ps = ps.tile([64, 392], F32, tag="mm")
            for cc in range(CC):
                nc.tensor.matmul(tp_ps,
                                 lhsT=wrT_t[:, cc, :],
                                 rhs=xcr[:, cc, h * 392:(h + 1) * 392],
                                 start=(cc == 0), stop=(cc == CC - 1))
            nc.scalar.activation(tpl_sb[:, h * 392:(h + 1) * 392], tp_ps, AF.Copy)
        tmx = pers.tile([64, 8], F32, tag="tmx")
        tmi = pers.tile([64, 8], U32, tag="tmi")
        nc.vector.max(out=tmx, in_=tpl_sb)
        nc.vector.max_index(out=tmi, in_max=tmx, in_values=tpl_sb)
        spt_f = pers.tile([64, 1], F32, tag="sptf")
        nc.vector.tensor_copy(spt_f, tmi[:, 0:1])

        # wrapped idx layout on-chip: transpose to one partition, reorder
        # free dim to (p16, rh), spread to 16 partitions with one small DMA
        tp_s = pstp.tile([1, 512], F32, tag="tp")
        nc.tensor.transpose(tp_s[:, 0:64], spt_f, idf_t[0:64, 0:64])
        t1s = pers.tile([1, 64], I16, tag="t1s")
        nc.vector.tensor_copy(
            t1s.rearrange("one (p r) -> one p r", p=16),
            tp_s[:, 0:64].rearrange("one (r p) -> one p r", p=16))
        w64 = pers.tile([16, 4], I16, tag="w64")
        nc.scalar.dma_start(out=w64,
                            in_=t1s.rearrange("one (p r) -> one p r", p=16))
        gidx64 = pers.tile([128, 4], I16, tag="gidx64")
        for g in range(8):
            nc.scalar.dma_start(out=gidx64[16 * g:16 * (g + 1), :], in_=w64)
        tresT = pers.tile([128, 1, CE], F32, tag="tresT")
        gthr_inst = nc.gpsimd.dma_gather(
            out_ap=tresT, in_ap=X_SC, idxs_ap=gidx64,
            num_idxs=64, num_idxs_reg=64, elem_size=CE)
        # rest of x loads AFTER the tres staging so its small DMAs are not
        # stuck behind megabytes of bulk traffic in the shared SDMA rings
        from concourse.tile_rust import add_dep_helper
        first_tb = None
        for tb in range(1, T):
            for hh in range(2):
                i0 = tb * HW + hh * 392
                dma = nc.sync.dma_start(out=xc[:, :, i0:i0 + 392],
                                        in_=Xrr[:, :, i0:i0 + 392])
                if first_tb is None:
                    first_tb = dma
                    try:
                        add_dep_helper(dma.ins, gthr_inst.ins, sync=True,
                                       reason="bulk x loads after idx staging")
                    except AttributeError:
                        add_dep_helper(dma, gthr_inst, sync=True,
                                       reason="bulk x loads after idx staging")
        tres = pers.tile([128, CC, R], F32R, tag="tres")
        for cc in range(CC):
            tp = pstp.tile([128, 512], F32, tag="tp")
            nc.tensor.transpose(tp[:, 0:64],
                                tresT[0:64, 0, cc * 128:(cc + 1) * 128],
                                idf_t[0:64, 0:64])
            nc.scalar.activation(tres[:, cc, :], tp[:, 0:64], AF.Copy)

        # ---- phase 2: affinity + topk (per t; 64-partition tiles) ----
        # staging layout: gstage2[r, (k t)] so that dram j = 512k + 64t + r
        gstage2 = pers.tile([64, 32], F32, tag="gstage2")
        gs2_v = gstage2.rearrange("r (k t) -> r t k", t=8)
        tres_mm = tres if AFF_F32R else tres.bitcast(F32)
        xc_mm = xcr if AFF_F32R else xc.bitcast(F32)
        for t in range(T):
            aff_sb = sb.tile([64, HW], F32, tag="aff")
            for h in range(2):
                a_ps = ps.tile([64, 392], F32, tag="mm")
                for cc in range(CC):
                    nc.tensor.matmul(
                        a_ps,
                        lhsT=tres_mm[:, cc, :],
                        rhs=xc_mm[:, cc, t * HW + h * 392: t * HW + (h + 1) * 392],
                        start=(cc == 0), stop=(cc == CC - 1))
                nc.scalar.activation(aff_sb[:, h * 392:(h + 1) * 392], a_ps, AF.Copy)
            amx = sb.tile([64, 8], F32, tag="amx")
            ami = sb.tile([64, 8], U32, tag="ami")
            nc.vector.max(out=amx, in_=aff_sb)
            nc.vector.max_index(out=ami, in_max=amx, in_values=aff_sb)
            nc.vector.tensor_scalar(gs2_v[:, t, :], ami[:, 0:K],
                                    float(t * HW), None, op0=ALU.add)

        # build wrapped idx layout on-chip: gstage2[r=16rh+p16, q=8k+t]
        # -> w16[p16, 4q+rh] via PE int16 transposes, then replicate 8x.
        t1_ps = pstp.tile([32, 64], F32, tag="tp")
        nc.tensor.transpose(t1_ps, gstage2, idf_t[0:64, 0:64])
        t1 = pers.tile([32, 64], F32, tag="t1")
        nc.vector.tensor_copy(t1, t1_ps)
        w16 = pers.tile([16, 128], I16, tag="w16")
        w16v = w16.rearrange("p (q rh) -> p q rh", rh=4)
        for rh in range(4):
            wr_ps = pstp.tile([16, 32], F32, tag="tp")
            nc.tensor.transpose(wr_ps, t1[:, 16 * rh:16 * (rh + 1)],
                                idf_t[0:32, 0:32])
            nc.vector.tensor_copy(w16v[:, :, rh], wr_ps)
        gidx2 = pers.tile([128, 128], I16, tag="gidx2")
        for g in range(8):
            nc.scalar.dma_start(out=gidx2[16 * g:16 * (g + 1), :], in_=w16)

        # ---- phase 3: traj gathers, fuse, points, conv ----
        fm_f32 = pers.tile([128, P], F32, tag="fmf")
        spts = pers.tile([128, 4, P], F32, tag="spts")
        gk = []
        for k in range(K):
            gk_t = sb.tile([128, 4, CE], F32, tag="gk")
            gk.append(gk_t)
            nc.gpsimd.dma_gather(out_ap=gk_t, in_ap=X_SC,
                                 idxs_ap=gidx2[:, k * 32:(k + 1) * 32],
                                 num_idxs=512, num_idxs_reg=512, elem_size=CE)
            trajk = sb.tile([128, CC, P], BF16, tag="trajk")
            for cc in range(CC):
                tp = pstp.tile([128, 512], F32, tag="tp")
                for jb in range(4):
                    nc.tensor.transpose(tp[:, jb * 128:(jb + 1) * 128],
                                        gk_t[:, jb, cc * 128:(cc + 1) * 128], idf_t)
                nc.scalar.activation(trajk[:, cc, :], tp, AF.Copy)
            # coords rows (gathered cols 512:514) -> (2, P)
            cd = pstp.tile([2, 512], F32, tag="tp")
            for jb in range(4):
                nc.tensor.transpose(cd[:, jb * 128:(jb + 1) * 128],
                                    gk_t[:, jb, 512:514], idf_t)
            coordk = sb.tile([2, P], BF16, tag="coordk")
            nc.vector.tensor_copy(coordk, cd)
            f_ps = ps.tile([128, P], F32, tag="mm")
            for cc in range(CC):
                nc.tensor.matmul(f_ps, lhsT=wpT_t[:, cc, :], rhs=trajk[:, cc, :],
                                 start=(cc == 0), stop=False)
            nc.tensor.matmul(f_ps, lhsT=wpc_t, rhs=coordk,
                             start=False, stop=True)
            if k == 0:
                nc.scalar.activation(fm_f32, f_ps, AF.Copy)
            else:
                nc.vector.tensor_tensor(out=fm_f32, in0=fm_f32, in1=f_ps, op=ALU.max)
            if k == 1:
                nc.vector.tensor_tensor(out=spts, in0=gk[0][:, :, 0:P],
                                        in1=gk[1][:, :, 0:P], op=ALU.add)
            elif k > 1:
                nc.vector.tensor_tensor(out=spts, in0=spts,
                                        in1=gk_t[:, :, 0:P], op=ALU.add)
        fm = pers.tile([128, P], BF16, tag="fm")
        nc.vector.tensor_scalar(fm, fm_f32, b2_t, None, op0=ALU.add)

        # points = sum_k traj_k (fp32); 1/4 folded into exp scale
        pts_f = pers.tile([128, CC, P], F32R, tag="ptsf")
        for cc in range(CC):
            tp = pstp.tile([128, 512], F32, tag="tp")
            for jb in range(4):
                nc.tensor.transpose(tp[:, jb * 128:(jb + 1) * 128],
                                    spts[:, jb, cc * 128:(cc + 1) * 128], idf_t)
            nc.scalar.activation(pts_f[:, cc, :], tp, AF.Copy)
        ptsr = pts_f

        # conv over t (3 taps) + bias3 + relu -> tc bf16
        tc_bf = pers.tile([128, CC, P], BF16, tag="tcbf")
        for ct in range(CC):
            c_ps = ps.tile([128, P], F32, tag="mm")
            cs = slice(ct * 128, (ct + 1) * 128)
            nc.tensor.matmul(c_ps, lhsT=wtT_t[:, 1, cs], rhs=fm,
                             start=True, stop=False)
            nc.tensor.matmul(c_ps[:, R:P], lhsT=wtT_t[:, 0, cs], rhs=fm[:, 0:P - R],
                             start=False, stop=False)
            nc.tensor.matmul(c_ps[:, 0:P - R], lhsT=wtT_t[:, 2, cs], rhs=fm[:, R:P],
                             start=False, stop=True)
            nc.scalar.activation(tc_bf[:, ct, :], c_ps, AF.Relu,
                                 bias=b3_t[:, ct:ct + 1])
        # tcT: (p, c) layout for prop matmuls
        tcT = pers.tile([128, CC, C], BF16, tag="tcT")
        for pb in range(4):
            tp2 = pstp.tile([128, 512], BF16, tag="tp")
            for cc in range(CC):
                nc.tensor.transpose(tp2[:, cc * 128:(cc + 1) * 128],
                                    tc_bf[:, cc, pb * 128:(pb + 1) * 128], idb_t)
            nc.vector.tensor_copy(tcT[:, pb, :], tp2)

        # ---- phase 4: z (s,p) -> softmax -> prop -> out, per s-tile ----
        # z per s-tile: lhsT = x slices, rhs = pts (f32r).  exp with
        # per-tile max bias + accumulated denominator; e transposed via PE
        # (bf16) and fed back as prop lhsT; normalize + residual fused in
        # one DVE pass at the output.
        for i in range(NCH):
            cw = min(P, S - i * P)
            nt = cw // 128
            xres = sb.tile([128, 4, C], F32, tag="xres")
            nc.sync.dma_start(out=xres[:, 0:nt, :],
                              in_=XSCr[:, 4 * i:4 * i + nt, 0:C])
            for t in range(nt):
                st = 4 * i + t
                z_ps = ps.tile([128, P], F32, tag="mm")
                for cc in range(CC):
                    nc.tensor.matmul(z_ps,
                                     lhsT=xcr[:, cc, st * 128:(st + 1) * 128],
                                     rhs=ptsr[:, cc, :],
                                     start=(cc == 0), stop=(cc == CC - 1))
                nm = sb3.tile([128, 1], F32, tag="nm")
                nc.vector.tensor_reduce(nm, z_ps, axis=AX.X, op=ALU.max,
                                        negate=True)
                nm4 = sb3.tile([128, 1], F32, tag="nm4")
                nc.vector.tensor_scalar(nm4, nm, 0.25, None, op0=ALU.mult)
                e_sb = sb3.tile([128, P], BF16, tag="esb")
                dsum = sb3.tile([128, 1], F32, tag="dsum")
                nc.scalar.activation(e_sb, z_ps, AF.Exp, bias=nm4, scale=0.25,
                                     accum_out=dsum)
                eT_ps = pstp.tile([128, 512], BF16, tag="tp")
                for pb in range(4):
                    nc.tensor.transpose(eT_ps[:, pb * 128:(pb + 1) * 128],
                                        e_sb[:, pb * 128:(pb + 1) * 128], idb_t)
                eT = sb3.tile([128, 512], BF16, tag="eT")
                nc.scalar.activation(eT, eT_ps, AF.Copy)
                pr = pspr.tile([128, C], F32, tag="pr")
                for pb in range(4):
                    nc.tensor.matmul(pr, lhsT=eT[:, pb * 128:(pb + 1) * 128],
                                     rhs=tcT[:, pb, :],
                                     start=(pb == 0), stop=(pb == CC - 1))
                rd = sb3.tile([128, 1], F32, tag="rd")
                nc.vector.reciprocal(rd, dsum)
                osb = sb3.tile([128, C], F32, tag="osb")
                nc.vector.scalar_tensor_tensor(
                    osb, in0=pr, scalar=rd, in1=xres[:, t, :],
                    op0=ALU.mult, op1=ALU.add)
                nc.sync.dma_start(out=OUTr[:, st, :], in_=osb)
        ctx.close()
    nc.compile()
    return nc


def _host_prep(inputs):
    eps = 1e-5
    f32 = np.float32
    import ml_dtypes
    bf16 = ml_dtypes.bfloat16
    x = np.asarray(inputs["input"], f32)                       # (B,C,T,H,W)
    s1 = np.asarray(inputs["bn1_gamma"]) / np.sqrt(np.asarray(inputs["bn1_var"]) + eps)
    wrT = (np.asarray(inputs["w_reduce"], f32) * s1[:, None]).T.astype(f32)
    s2 = np.asarray(inputs["bn2_gamma"]) / np.sqrt(np.asarray(inputs["bn2_var"]) + eps)
    wp = np.asarray(inputs["w_proj"], f32) * s2[:, None]       # (Cq, C+2)
    b2 = (np.asarray(inputs["bn2_beta"])
          - np.asarray(inputs["bn2_mean"]) * s2).astype(f32)
    s3 = np.asarray(inputs["bn3_gamma"]) / np.sqrt(np.asarray(inputs["bn3_var"]) + eps)
    wt = np.asarray(inputs["w_t"], f32)[:, :, :, 0] * s3[:, None, None]  # (C,Cq,3)
    b3 = (np.asarray(inputs["bn3_beta"])
          - np.asarray(inputs["bn3_mean"]) * s3).astype(f32)
    common = {
        "wrT": np.ascontiguousarray(wrT),
        "wpT": np.ascontiguousarray(wp[:, :C].T.astype(bf16)),
        "wpc": np.ascontiguousarray(wp[:, C:].T.astype(bf16)),
        "wtT": np.ascontiguousarray(np.transpose(wt, (2, 1, 0)).astype(bf16)),
        "b2": b2.reshape(Cq, 1),
        "b3": b3.reshape(CC, 128),
        "identbf": np.eye(128, dtype=bf16),
        "identf": np.eye(128, dtype=f32),
    }
    x_cs = x.reshape(B, C, S)
    # augmented (S, CE) per-sample: x^T | row/H | col/W | zero pad
    hw_idx = np.arange(HW, dtype=f32)
    rowv = np.tile((hw_idx // W) / H, T)                       # (S,)
    colv = np.tile((hw_idx % W) / W, T)
    in_maps = []
    for b in range(B):
        m = dict(common)
        m["x_cs"] = np.ascontiguousarray(x_cs[b])
        aug = np.zeros((S, CE), f32)
        aug[:, :C] = x_cs[b].T
        aug[:, C] = rowv
        aug[:, C + 1] = colv
        m["x_sc"] = aug
        in_maps.append(m)
    return in_maps


def kernel(**inputs) -> np.ndarray:
    if "nc" not in _CACHED:
        _CACHED["nc"] = build_nc()
    nc = _CACHED["nc"]
    in_maps = _host_prep(inputs)
    res = run_bass_kernel_spmd(nc, in_maps, list(range(B)))
    out = np.stack([res.results[b]["out_sc"].T for b in range(B)], axis=0)
    return out.reshape(B, C, T, H, W).astype(np.float32)
